# revision 17
# baseline (speedup 1.0000x reference)
"""Complex multi-head attention on 8 Trainium2 cores (Bass/Tile).

Sharding: pure data-parallel over batch (B=8 -> 1 batch per core),
weights replicated. No collectives.

v2 design (vs baseline):
  - Q/K path fp32r (exact scores), everything else bf16. PE column rate
    is identical for fp32r/bf16 at N=512, so bf16 buys DMA/SBUF/DVE
    only where precision allows.
  - Row sums no longer burn 256 PE matmuls: exp tiles accumulate across
    key chunks on the DVE (bf16 adds) and are reduced with ONE
    ones-matmul pair per (head, query-half) group.
  - Softmax reciprocal on the ACT engine as exp(-ln(x)) (both funcs live
    in the natural_log_exp_and_others table set; the banned Reciprocal
    table is never used).
  - Scores PSUM tiles are [128,1024] (2 banks): both complex components
    of one key chunk share a tile, so exp is ONE wide ACT op.
  - All inputs stream up front in dedicated pools; software-pipelined PE
    stream weaves head h+1's Q/K projection matmuls into head h's
    attention groups so the PE never idles on the exp chain.
"""

import sys
import types
import numpy as np
from ml_dtypes import bfloat16

B, S, D, H = 8, 1024, 512, 8
DH = D // H
KC = 8  # k-chunks of 128 over (c,d) = 1024
TC = 8  # token chunks of 128
NCORES = 8
LAG = 2  # AV matmul lag behind scores (exp latency cover)

LAST_EXEC_NS = None


# ---------------------------------------------------------------- shims
def _install_axon_profile_shim():
    if "antenv.axon_hooks" in sys.modules:
        return
    try:
        import antenv  # noqa: F401

        mod = types.ModuleType("antenv.axon_hooks")
        state = {"hook": None}
        mod.set_axon_ntff_profile_hook = lambda h: state.__setitem__("hook", h)
        mod.get_axon_ntff_profile_hook = lambda: state["hook"]
        sys.modules["antenv.axon_hooks"] = mod
        from trn_agent_boot.trn_boot import _ntff_profile_via_ctypes

        hook = _ntff_profile_via_ctypes("/opt/axon/libaxon_pjrt.so")
        if hook is not None:
            mod.set_axon_ntff_profile_hook(hook)
    except Exception:
        pass


def _install_tile_drain_patch():
    """This walrus build allows ONE sync wait per instruction; split the
    TileContext exit drain's waits across preceding sync NOPs."""
    import concourse.mybir as mybir
    import concourse.tile as tile
    from concourse.vector_clock import ScopedClock

    if getattr(tile.TileContext, "_drain_patched", False):
        return

    def _patched(self, tick_clock, wait_clock):
        probe = mybir.InstNoOp(name="I-drain-probe")
        probe.engine = mybir.EngineType.SP
        wait_clock.add_sem_waits(probe, ScopedClock({None: tick_clock.global_clock}))
        waits = list(probe.sync_info.on_wait or []) if probe.sync_info else []
        for w in waits:
            nop = self.nc.sync.nop()
            nop.ins.sync_info = mybir.SyncInfo(on_wait=[w], on_update=[])
        self.nc.sync.drain()
        self.nc.all_engine_barrier()
        assert self.sems is not None
        popped = self.nc._tile_sem_poison_stack.pop()
        assert popped is self._sem_poison
        self.nc.clear_and_free_semaphores(list(self.sems.allocated().values()))
        self.nc.all_engine_barrier()

    tile.TileContext._drain_and_barrier = _patched
    tile.TileContext._drain_patched = True


def _split_waits(nc, max_waits=1):
    """Hoist extra sync waits onto preceding same-engine NOPs (walrus here
    rejects >1 sync wait per instruction)."""
    import concourse.mybir as mybir

    def process(blk):
        lst = blk.instructions
        i = 0
        while i < len(lst):
            inst = lst[i]
            if hasattr(inst, "blocks"):
                for b in inst.blocks or []:
                    process(b)
            si = inst.sync_info
            if si is not None and si.on_wait and len(si.on_wait) > max_waits:
                waits = list(si.on_wait)
                keep, extra = waits[-max_waits:], waits[:-max_waits]
                inst.sync_info = mybir.SyncInfo(
                    on_wait=keep, on_update=list(si.on_update or [])
                )
                for j, w in enumerate(extra):
                    nop = mybir.InstNoOp(name=f"{inst.name}-ws{j}")
                    nop.engine = inst.engine
                    nop.sync_info = mybir.SyncInfo(on_wait=[w], on_update=[])
                    lst.insert(i, nop)
                    i += 1
            i += 1

    for f in nc.m.functions:
        for blk in f.blocks:
            process(blk)


# ------------------------------------------------------------ host prep
def _build_wqk(wr, wi, scale):
    """[1024 k=(c,d), 1024 m=(h, c', dh)] for Q/K/V projections."""
    W = np.empty((2 * D, 2 * D), np.float32)
    for h in range(H):
        o = slice(h * DH, (h + 1) * DH)
        c0 = h * 2 * DH
        W[0:D, c0 : c0 + DH] = wr[o].T * scale
        W[D:, c0 : c0 + DH] = -wi[o].T * scale
        W[0:D, c0 + DH : c0 + 2 * DH] = wi[o].T * scale
        W[D:, c0 + DH : c0 + 2 * DH] = wr[o].T * scale
    return W


def _head_tiles(W):
    """[1024,1024] -> [H, 128, 1024]: per-head column block, k-chunk cols."""
    out = np.empty((H, 128, 1024), np.float32)
    for h in range(H):
        blk = W[:, h * 128 : (h + 1) * 128]  # [1024, 128]
        for kk in range(KC):
            out[h, :, kk * 128 : (kk + 1) * 128] = blk[kk * 128 : (kk + 1) * 128]
    return out


def _kchunk_tiles(W):
    """[1024,1024] -> [KC, 128, 1024]: row chunks."""
    return np.ascontiguousarray(W.reshape(KC, 128, 1024))


def _build_wo(wo_r, wo_i):
    """rows (h, c', dh), cols (o, c) interleaved to match [S, D, 2]."""
    W = np.empty((2 * D, 2 * D), np.float32)
    for h in range(H):
        d = slice(h * DH, (h + 1) * DH)
        r0 = h * 2 * DH
        W[r0 : r0 + DH, 0::2] = wo_r[:, d].T
        W[r0 : r0 + DH, 1::2] = wo_i[:, d].T
        W[r0 + DH : r0 + 2 * DH, 0::2] = -wo_i[:, d].T
        W[r0 + DH : r0 + 2 * DH, 1::2] = wo_r[:, d].T
    return W


def _xt(x):  # [S, D, 2] -> [2D, S] feature-major
    out = np.empty((2 * D, S), np.float32)
    out[0:D] = x[:, :, 0].T
    out[D:] = x[:, :, 1].T
    return out


# ------------------------------------------------------------ bass build
def _build_nc():
    import concourse.bass as bass
    import concourse.mybir as mybir
    import concourse.tile as tile
    from contextlib import ExitStack

    MDT = mybir.dt.float32r
    F32 = mybir.dt.float32
    BF16 = mybir.dt.bfloat16
    Exp = mybir.ActivationFunctionType.Exp
    Ln = mybir.ActivationFunctionType.Ln

    nc = bass.Bass()
    d_xtq = nc.dram_tensor("xtq", [KC, 128, S], BF16, kind="ExternalInput")
    d_xtk = nc.dram_tensor("xtk", [KC, 128, S], BF16, kind="ExternalInput")
    d_xtv = nc.dram_tensor("xtv", [KC, 128, S], BF16, kind="ExternalInput")
    d_wq = nc.dram_tensor("wq", [H, 128, 1024], BF16, kind="ExternalInput")
    d_wk = nc.dram_tensor("wk", [H, 128, 1024], BF16, kind="ExternalInput")
    d_wv = nc.dram_tensor("wv", [KC, 128, 1024], BF16, kind="ExternalInput")
    d_wo = nc.dram_tensor("wo", [H, 128, 1024], BF16, kind="ExternalInput")
    d_ones = nc.dram_tensor("ones", [128, 128], BF16, kind="ExternalInput")
    d_sgn = nc.dram_tensor("sgn", [128, 1], F32, kind="ExternalInput")
    d_out = nc.dram_tensor("out", [S, 1024], F32, kind="ExternalOutput")

    with tile.TileContext(nc) as tc, ExitStack() as ctx:
        ctx.enter_context(
            nc.allow_low_precision(reason="bf16 activations within tolerance")
        )
        # ---- SBUF pools ----
        pXtv = ctx.enter_context(tc.tile_pool(name="xtv", bufs=8))  # -> osb
        pWv = ctx.enter_context(tc.tile_pool(name="wv", bufs=8))  # -> wo
        pXq = ctx.enter_context(tc.tile_pool(name="xq", bufs=8))  # -> oev
        pXk = ctx.enter_context(tc.tile_pool(name="xk", bufs=8))
        pWqk = ctx.enter_context(tc.tile_pool(name="wqk", bufs=3))
        pV1 = ctx.enter_context(tc.tile_pool(name="v1", bufs=8))
        pV2 = ctx.enter_context(tc.tile_pool(name="v2", bufs=8))
        pStk = ctx.enter_context(tc.tile_pool(name="stk", bufs=8))
        pE = ctx.enter_context(tc.tile_pool(name="e", bufs=4))
        pEacc = ctx.enter_context(tc.tile_pool(name="eacc", bufs=4))
        pRec = ctx.enter_context(tc.tile_pool(name="rec", bufs=4))
        pC = ctx.enter_context(tc.tile_pool(name="const", bufs=1))

        # ---- PSUM pools (8 banks: 2x2 + 2x1 + 1x2) ----
        ps_st = ctx.enter_context(tc.tile_pool(name="ps_st", bufs=2, space="PSUM"))
        ps_p12 = ctx.enter_context(tc.tile_pool(name="ps_p12", bufs=2, space="PSUM"))
        ps_misc = ctx.enter_context(tc.tile_pool(name="ps_misc", bufs=1, space="PSUM"))

        # ---- constants + input DMAs (priority order) ----
        ones = pC.tile([128, 128], BF16, tag="ones")
        nc.sync.dma_start(out=ones, in_=d_ones[:, :])
        sgn = pC.tile([128, 1], F32, tag="sgn")
        nc.sync.dma_start(out=sgn, in_=d_sgn[:, :])

        # Input streams: each tile splits into two half-partition DMAs;
        # SP issues the top halves and Pool the bottom halves, both in
        # need order (xtv/wv -> xtq/wq0 -> xtk/wk0), so transfers start
        # in priority order across the DMA rings.
        xtv, wv, xtq, xtk = [], [], [], []
        pairs = []
        for kk in range(KC):
            tv = pXtv.tile([128, S], BF16, tag="big", name=f"xtv{kk}")
            xtv.append(tv)
            tw = pWv.tile([128, 1024], BF16, tag="wv", name=f"wv{kk}")
            wv.append(tw)
            pairs.append((tv, d_xtv[kk]))
            pairs.append((tw, d_wv[kk]))
        wq = {0: pWqk.tile([128, 1024], BF16, tag="wqk", name="wq0")}
        wk = {0: pWqk.tile([128, 1024], BF16, tag="wqk", name="wk0")}
        for kk in range(KC):
            t = pXq.tile([128, S], BF16, tag="xq", name=f"xtq{kk}")
            xtq.append(t)
            pairs.append((t, d_xtq[kk]))
        pairs.append((wq[0], d_wq[0]))
        for kk in range(KC):
            t = pXk.tile([128, S], BF16, tag="xk", name=f"xtk{kk}")
            xtk.append(t)
            pairs.append((t, d_xtk[kk]))
        pairs.append((wk[0], d_wk[0]))
        for tile_, dram2d in pairs:
            nc.sync.dma_start(out=tile_[0:64, :], in_=dram2d[0:64, :])
        for tile_, dram2d in pairs:
            nc.gpsimd.dma_start(out=tile_[64:128, :], in_=dram2d[64:128, :])

        # ---- phase 1: V projection + v2 prep ----
        # kk-outer over rounds of 3 token chunks: the PE consumes each
        # xtv/wv chunk as its DMA lands instead of stalling per chunk
        v1, v2 = [None] * TC, [None] * TC
        for rnd in ([0, 1, 2], [3, 4, 5], [6, 7]):
            pss = {}
            for ri, t_ in enumerate(rnd):
                pool = ps_misc if ri == 2 else ps_st
                pss[t_] = pool.tile(
                    [128, 1024], F32, tag=pool.name, name=f"psv{t_}"
                )
            for kk in range(KC):
                for t_ in rnd:
                    for nh in range(2):
                        nsl = slice(nh * 512, (nh + 1) * 512)
                        nc.tensor.matmul(
                            pss[t_][:, nsl],
                            lhsT=xtv[kk][:, t_ * 128 : (t_ + 1) * 128],
                            rhs=wv[kk][:, nsl],
                            start=(kk == 0),
                            stop=(kk == KC - 1),
                        )
            for t_ in rnd:
                vt = pV1.tile([128, 1024], BF16, tag="v1", name=f"v1_{t_}")
                nc.scalar.copy(vt, pss[t_])
                v1[t_] = vt
                v2t = pV2.tile([128, 1024], BF16, tag="v2", name=f"v2_{t_}")
                for h in range(H):
                    b = h * 128
                    nc.vector.tensor_scalar_mul(
                        v2t[:, b : b + 64], vt[:, b + 64 : b + 128], -1.0
                    )
                    nc.vector.tensor_copy(v2t[:, b + 64 : b + 128], vt[:, b : b + 64])
                v2[t_] = v2t

        # WO streams during attention into the freed wv slots
        wo = []
        for h in range(H):
            t = pWv.tile([128, 1024], BF16, tag="wv", name=f"wo{h}")
            nc.sync.dma_start(out=t, in_=d_wo[h])
            wo.append(t)

        # ---- Q/K projection generators (woven into attention groups) ----
        qtiles = {}  # h -> (qneg, qswap)
        ktiles = {}  # h -> kstack

        def qproj_gen(h, pool):
            wqh = wq[h]
            qstack = pStk.tile([128, S], BF16, tag="stk", name=f"qstack{h}")
            qneg = pStk.tile([128, S], BF16, tag="stk", name=f"qneg{h}")
            qswap = pStk.tile([128, S], BF16, tag="stk", name=f"qswap{h}")
            qtiles[h] = (qneg, qswap)
            for nh in range(2):
                nsl = slice(nh * 512, (nh + 1) * 512)
                ps = pool.tile([128, 512], F32, tag=pool.name, name=f"psq{h}{nh}")
                for kk in range(KC):
                    nc.tensor.matmul(
                        ps,
                        lhsT=wqh[:, kk * 128 : (kk + 1) * 128],
                        rhs=xtq[kk][:, nsl],
                        start=(kk == 0),
                        stop=(kk == KC - 1),
                    )
                    yield
                nc.vector.tensor_copy(qstack[:, nsl], ps)
                nc.vector.tensor_scalar_mul(qneg[:, nsl], ps, sgn)
                # qswap = [qi; qr] via partition-crossing SBUF->SBUF DMA
                nc.sync.dma_start(out=qswap[0:64, nsl], in_=qstack[64:128, nsl])
                nc.sync.dma_start(out=qswap[64:128, nsl], in_=qstack[0:64, nsl])

        def kproj_gen(h, pool):
            wkh = wk[h]
            kstack = pStk.tile([128, S], BF16, tag="stk", name=f"kstack{h}")
            ktiles[h] = kstack
            for nh in range(2):
                nsl = slice(nh * 512, (nh + 1) * 512)
                ps = pool.tile([128, 512], F32, tag=pool.name, name=f"psk{h}{nh}")
                for kk in range(KC):
                    nc.tensor.matmul(
                        ps,
                        lhsT=wkh[:, kk * 128 : (kk + 1) * 128],
                        rhs=xtk[kk][:, nsl],
                        start=(kk == 0),
                        stop=(kk == KC - 1),
                    )
                    yield
                nc.vector.tensor_copy(kstack[:, nsl], ps)

        def drain(gen, n=None):
            if gen is None:
                return None
            try:
                if n is None:
                    for _ in gen:
                        pass
                    return None
                for _ in range(n):
                    next(gen)
                return gen
            except StopIteration:
                return None

        # ---- phase 2: head-0 projections (solid PE block) ----
        drain(qproj_gen(0, ps_st))
        drain(kproj_gen(0, ps_st))

        osb = []

        # group tail: row sums (partial eacc + e7 directly, so the last
        # DVE add is off the critical chain), reciprocal via ACT
        # exp(-ln(sums)), normalization fused into the p1/p2 eviction
        def finalize(h, nh, eacc, e7, p1, p2):
            nsl = slice(nh * 512, (nh + 1) * 512)
            sums = ps_misc.tile([128, 1024], F32, tag="ps_misc", name=f"sm{h}{nh}")
            for c, csl in ((0, slice(0, 512)), (1, slice(512, 1024))):
                nc.tensor.matmul(
                    sums[:, csl], lhsT=ones, rhs=eacc[:, csl], start=True, stop=False
                )
                nc.tensor.matmul(
                    sums[:, csl], lhsT=ones, rhs=e7[:, csl], start=False, stop=True
                )
            lns = pRec.tile([128, 1024], F32, tag="rec", name=f"ln{h}{nh}")
            nc.scalar.activation(lns, sums, func=Ln)
            rec = pRec.tile([128, 1024], F32, tag="rec", name=f"rc{h}{nh}")
            nc.scalar.activation(rec, lns, func=Exp, scale=-1.0)
            t1 = pRec.tile([128, 512], F32, tag="rec", name=f"t1{h}{nh}")
            t2 = pRec.tile([128, 512], F32, tag="rec", name=f"t2{h}{nh}")
            nc.vector.tensor_mul(t1, p1, rec[:, 0:512])
            nc.vector.tensor_mul(t2, p2, rec[:, 512:1024])
            nc.vector.tensor_add(osb[h][:, nsl], t1, t2)

        # ---- phase 3: attention groups with woven projections ----
        gen = None
        for h in range(H):
            ot = pXtv.tile([128, 1024], BF16, tag="big", name=f"osb{h}")
            osb.append(ot)
            qneg, qswap = qtiles[h]
            kstack = ktiles[h]
            for nh in range(2):
                nsl = slice(nh * 512, (nh + 1) * 512)
                if h < H - 1:
                    if nh == 0:
                        wq[h + 1] = pWqk.tile(
                            [128, 1024], BF16, tag="wqk", name=f"wq{h+1}"
                        )
                        nc.sync.dma_start(out=wq[h + 1], in_=d_wq[h + 1])
                        gen = qproj_gen(h + 1, ps_misc)
                    else:
                        wk[h + 1] = pWqk.tile(
                            [128, 1024], BF16, tag="wqk", name=f"wk{h+1}"
                        )
                        nc.sync.dma_start(out=wk[h + 1], in_=d_wk[h + 1])
                        gen = kproj_gen(h + 1, ps_misc)
                else:
                    gen = None

                p1 = ps_p12.tile([128, 512], F32, tag="ps_p12", name=f"p1_{h}{nh}")
                p2 = ps_p12.tile([128, 512], F32, tag="ps_p12", name=f"p2_{h}{nh}")
                etiles = []
                eacc = None
                # adds into eacc (== e0) must be emitted AFTER the lagged
                # AV matmul that reads the pristine e0 (program order is
                # dependency order for the Tile framework)
                pending_adds = []
                for i in range(TC):
                    ksl = slice(i * 128, (i + 1) * 128)
                    st = ps_st.tile([128, 1024], F32, tag="ps_st", name=f"s{h}{nh}{i}")
                    nc.tensor.matmul(
                        st[:, 0:512],
                        lhsT=kstack[:, ksl],
                        rhs=qneg[:, nsl],
                        start=True,
                        stop=True,
                    )
                    nc.tensor.matmul(
                        st[:, 512:1024],
                        lhsT=kstack[:, ksl],
                        rhs=qswap[:, nsl],
                        start=True,
                        stop=True,
                    )
                    if i == 0:
                        e = pEacc.tile([128, 1024], BF16, tag="eacc", name=f"ea{h}{nh}")
                        eacc = e
                    else:
                        e = pE.tile([128, 1024], BF16, tag="e", name=f"e{h}{nh}{i}")
                    nc.scalar.activation(e, st, func=Exp)
                    etiles.append(e)
                    if 1 <= i <= TC - 2:
                        # e7 goes straight into the sums matmul instead
                        if i <= LAG:
                            pending_adds.append(e)
                        else:
                            nc.vector.tensor_add(eacc, eacc, e)
                    if i in (2, 4):
                        gen = drain(gen, 9)
                    if i >= LAG:
                        j = i - LAG
                        nc.tensor.matmul(
                            p1,
                            lhsT=v1[j][:, h * 128 : (h + 1) * 128],
                            rhs=etiles[j][:, 0:512],
                            start=(j == 0),
                            stop=(j == TC - 1),
                        )
                        nc.tensor.matmul(
                            p2,
                            lhsT=v2[j][:, h * 128 : (h + 1) * 128],
                            rhs=etiles[j][:, 512:1024],
                            start=(j == 0),
                            stop=(j == TC - 1),
                        )
                        if j == 0:
                            for pe in pending_adds:
                                nc.vector.tensor_add(eacc, eacc, pe)
                            pending_adds = []
                for j in range(TC - LAG, TC):
                    nc.tensor.matmul(
                        p1,
                        lhsT=v1[j][:, h * 128 : (h + 1) * 128],
                        rhs=etiles[j][:, 0:512],
                        start=(j == 0),
                        stop=(j == TC - 1),
                    )
                    nc.tensor.matmul(
                        p2,
                        lhsT=v2[j][:, h * 128 : (h + 1) * 128],
                        rhs=etiles[j][:, 512:1024],
                        start=(j == 0),
                        stop=(j == TC - 1),
                    )
                finalize(h, nh, eacc, etiles[TC - 1], p1, p2)
                drain(gen)
                gen = None

        # ---- phase 4: output projection ----
        for t_ in range(TC):
            oev = pXq.tile([128, 1024], F32, tag="xq", name=f"oev{t_}")
            ps = ps_st.tile([128, 1024], F32, tag="ps_st", name=f"pso{t_}")
            for nh in range(2):
                nsl = slice(nh * 512, (nh + 1) * 512)
                for h in range(H):
                    nc.tensor.matmul(
                        ps[:, nsl],
                        lhsT=osb[h][:, t_ * 128 : (t_ + 1) * 128],
                        rhs=wo[h][:, nsl],
                        start=(h == 0),
                        stop=(h == H - 1),
                    )
                nc.scalar.copy(oev[:, nsl], ps[:, nsl])
                # finer splits toward the tail so the last transfers
                # spread across rings instead of serializing the finish
                npc = 2 if t_ < TC - 2 else 4
                for p in range(npc):
                    w = 128 // npc
                    rsl = slice(t_ * 128 + p * w, t_ * 128 + (p + 1) * w)
                    psl = slice(p * w, (p + 1) * w)
                    eng = nc.sync if p % 2 == 0 else nc.gpsimd
                    eng.dma_start(out=d_out[rsl, nsl], in_=oev[psl, nsl])

    _split_waits(nc)
    return nc


_NC_CACHE = {}


def kernel(
    queries,
    keys,
    values,
    wq_r,
    wq_i,
    wk_r,
    wk_i,
    wv_r,
    wv_i,
    wo_r,
    wo_i,
    _trace=False,
):
    global LAST_EXEC_NS
    _install_axon_profile_shim()
    _install_tile_drain_patch()
    from concourse.bass_utils import run_bass_kernel_spmd

    scale = 1.0 / np.sqrt(DH)
    WQ = _head_tiles(_build_wqk(np.asarray(wq_r), np.asarray(wq_i), scale)).astype(
        bfloat16
    )
    WK = _head_tiles(_build_wqk(np.asarray(wk_r), np.asarray(wk_i), 1.0)).astype(
        bfloat16
    )
    WV = _kchunk_tiles(_build_wqk(np.asarray(wv_r), np.asarray(wv_i), 1.0)).astype(
        bfloat16
    )
    WO = _kchunk_tiles(_build_wo(np.asarray(wo_r), np.asarray(wo_i))).astype(bfloat16)
    ONES = np.ones((128, 128), bfloat16)
    SGN = np.ones((128, 1), np.float32)
    SGN[64:] = -1.0

    queries = np.asarray(queries)
    keys = np.asarray(keys)
    values = np.asarray(values)

    in_maps = []
    for b in range(NCORES):
        in_maps.append(
            {
                "xtq": _xt(queries[b]).reshape(KC, 128, S).astype(bfloat16),
                "xtk": _xt(keys[b]).reshape(KC, 128, S).astype(bfloat16),
                "xtv": _xt(values[b]).reshape(KC, 128, S).astype(bfloat16),
                "wq": WQ,
                "wk": WK,
                "wv": WV,
                "wo": WO,
                "ones": ONES,
                "sgn": SGN,
            }
        )

    if "nc" not in _NC_CACHE:
        _NC_CACHE["nc"] = _build_nc()
    nc = _NC_CACHE["nc"]

    res = run_bass_kernel_spmd(nc, in_maps, list(range(NCORES)), trace=_trace)
    LAST_EXEC_NS = res.exec_time_ns

    out = np.empty((B, S, D, 2), np.float32)
    for b in range(NCORES):
        out[b] = res.results[b]["out"].reshape(S, D, 2)
    return out
